# revision 23
# baseline (speedup 1.0000x reference)
"""MetaBaseline (retrieval_knn) Trainium2 kernel — bf16 pipeline.

Computation (per episode b):
  q  = l2norm(input1[b])            # [75, 25, 640] over channel
  s  = l2norm(input2[b])            # [5, 5, 25, 640]
  att = softmax_hw(s @ rpn_w)       # rpn_b is softmax-invariant
  cg  = leaky(sum_hw(att * s))
  feat = mean_shot(mean_hw(s) + 5 * cg)
  sim[b] = mean_hw(q) @ feat.T      # [75, 5]

Sharding: data-parallel over episodes, 4 per core on 8 cores.

Layout: spatial descriptors on SBUF partitions (125/tile, descriptor
d = 15p + j for query, 5p + j for support), channels on the free axis.
All inputs are pre-converted to bf16 on the host (halves DMA bytes and
engages the DVE 16-bit perf modes; PE streams bf16 at 1 col/cycle).
Every group-reduction over descriptors (hw-mean, softmax sums, weighted
channel attention) is a PE matmul against a small stationary mask with
per-descriptor weights folded in. The final sim GEMM contracts over
channels via PE transposes of qm and a direct featT product.
All episode DMAs are issued up-front (whole per-core input fits SBUF).
"""

import sys
from contextlib import ExitStack

sys.path.insert(0, "/opt/trn_rl_repo")

import numpy as np
import ml_dtypes

import concourse.bass as bass
import concourse.tile as tile
from concourse import bacc, mybir
from concourse.bass_utils import run_bass_kernel_spmd

F32 = mybir.dt.float32
BF16 = mybir.dt.bfloat16
I32 = mybir.dt.int32
OP = mybir.AluOpType
AF = mybir.ActivationFunctionType

# Problem constants (fixed by the problem statement).
B, QN, WAY, SHOT, HH, WW, C = 32, 75, 5, 5, 5, 5, 640
NCORES = 8
E = B // NCORES        # 4 episodes per core
HW = HH * WW           # 25 spatial positions
P = 125                # descriptors per tile
QT = 15                # query slots per partition (1875 = 125*15)
ST = 5                 # support slots per partition (625 = 125*5)
NMAP = WAY * SHOT      # 25 support maps / episode
GAMMA = 5.0
SLOPE = 0.01
NHI = 512              # psum-bank-sized column split
NLO = C - NHI          # 128

# Square-pass method per tile: "pow" = DVE tensor_scalar x^2 + accum (4x
# candidate), "tt" = DVE tensor_tensor square into scratch + tensor_scalar
# reduce, "stt" = DVE scalar_tensor_tensor (1x), "act" = ACT Square,
# "gp" = gpsimd scalar_tensor_tensor.
SQ_METHOD = {
    ("s", 0): "stt", ("s", 1): "act", ("s", 2): "tt", ("s", 3): "tt",
    ("s", 4): "tt",
    ("q", 0): "tt", ("q", 1): "tt", ("q", 2): "act", ("q", 3): "tt",
    ("q", 4): "tt", ("q", 5): "act", ("q", 6): "tt", ("q", 7): "tt",
    ("q", 8): "act", ("q", 9): "tt", ("q", 10): "act", ("q", 11): "tt",
    ("q", 12): "tt", ("q", 13): "act", ("q", 14): "tt",
}


def _build_body(ctx: ExitStack, tc: "tile.TileContext", i1, i2, rpnw, out):
    nc = tc.nc

    const_pool = ctx.enter_context(tc.tile_pool(name="const", bufs=1))
    data_pool = ctx.enter_context(tc.tile_pool(name="data", bufs=1))
    scr_pool = ctx.enter_context(tc.tile_pool(name="scratch", bufs=1))
    stats = ctx.enter_context(tc.tile_pool(name="stats", bufs=2))
    work = ctx.enter_context(tc.tile_pool(name="work", bufs=2))

    # PSUM budget is 8 banks of [128, 512] f32. Per episode (double
    # buffered): qm_hi bank, cgsm_hi bank, a shared "lo" bank holding both
    # 128-col tails, and a shared "smalls" bank (bf16 tile; f32 regions are
    # bitcast views) holding softmax sums, featT, qmT and sim.
    qm_ps = ctx.enter_context(tc.tile_pool(name="qmps", bufs=2, space="PSUM"))
    s_ps = ctx.enter_context(tc.tile_pool(name="sps", bufs=2, space="PSUM"))
    lo_ps = ctx.enter_context(tc.tile_pool(name="lops", bufs=2, space="PSUM"))
    small_ps = ctx.enter_context(tc.tile_pool(name="smallps", bufs=2, space="PSUM"))

    # rpn_w first, on the scalar HWDGE ring (ahead of any bulk)
    w_sb = const_pool.tile([1, C], BF16, name="w_sb", tag="w_sb")
    nc.scalar.dma_start(w_sb[:], rpnw)

    # ================= all bulk DMAs up-front =================
    qtiles, stiles = [], []
    for e in range(E):
        sb = data_pool.tile([P, ST * C], BF16, name=f"s_{e}", tag=f"s_{e}")
        qb = data_pool.tile([P, QT * C], BF16, name=f"q_{e}", tag=f"q_{e}")
        stiles.append(sb)
        qtiles.append(qb)
    # Ring plan: SWDGE (gpsimd, all 16 SDMA engines) carries ~7.2MB as
    # 7.2KB descriptors; the two HWDGE rings (sync=SP, scalar=ACT; ~5
    # engines each) carry ~2.8MB each. rpn_w goes FIRST on the scalar ring
    # so the softmax chain never queues behind bulk (the v1 mistake).
    hw_rings = [nc.sync, nc.scalar]
    for e in range(E):
        hw_rings[e % 2].dma_start(stiles[e][:], i2[e])
        nc.gpsimd.dma_start(qtiles[e][:, 0:3600], i1[e, :, 0:3600])
        nc.gpsimd.dma_start(qtiles[e][:, 3600:7200], i1[e, :, 3600:7200])
        hw_rings[(e + 1) % 2].dma_start(qtiles[e][:, 7200:9600],
                                        i1[e, :, 7200:9600])

    # ================= one-time constants =================
    QNP = 76  # query mask column stride (pad 75 -> 76: keeps per-slot
    #           slices 4-byte aligned so DVE picks the 4x perf mode)
    setup_f32 = scr_pool.tile([P, QT * QNP], F32, name="setup_f32", tag="setup")

    # query mask, all 15 slots: [125, 15, 76], value 1/25 where
    # 0 <= 15p + j - 25q <= 24
    def stair(dst_f32, ncols, slots, j, value):
        nc.gpsimd.memset(dst_f32, value)
        nc.gpsimd.affine_select(
            out=dst_f32, in_=dst_f32, pattern=[[-HW, ncols]],
            compare_op=OP.is_ge, fill=0.0, base=j, channel_multiplier=slots)
        nc.gpsimd.affine_select(
            out=dst_f32, in_=dst_f32, pattern=[[HW, ncols]],
            compare_op=OP.is_ge, fill=0.0, base=HW - 1 - j,
            channel_multiplier=-slots)

    nc.gpsimd.memset(setup_f32[:], 0.0)
    for j in range(QT):
        stair(setup_f32[:, j * QNP:j * QNP + QN], QN, QT, j, 1.0 / HW)
    qmask = const_pool.tile([P, QT * QNP], BF16, name="qmask", tag="qmask")
    nc.vector.tensor_copy(qmask[:], setup_f32[:])

    # support combined mask [125, 5 slots, 2, 32]: att part (1.0) on
    # map-cols 0-24, hw-mean part (1/25) on cols 32-56 (pad to 32 so the
    # mean rows land on psum partition 32 -- PSUM reads must be 32-aligned).
    NM2 = 32
    setup2 = scr_pool.tile([P, ST * 2 * NM2], F32, name="setup2", tag="setup2")
    nc.gpsimd.memset(setup2[:], 0.0)
    s2v = setup2[:].rearrange("p (j t m) -> p j t m", j=ST, t=2, m=NM2)
    for j in range(ST):
        stair(s2v[:, j, 0, 0:NMAP], NMAP, ST, j, 1.0)
        stair(s2v[:, j, 1, 0:NMAP], NMAP, ST, j, 1.0 / HW)
    stc = const_pool.tile([P, ST * 2 * NM2], BF16, name="stc", tag="stc")
    nc.vector.tensor_copy(stc[:], setup2[:])
    stcv = stc[:].rearrange("p (j t m) -> p j t m", j=ST, t=2, m=NM2)

    # shot-mean matrix [25 maps, 5 ways] (block diagonal, 1/SHOT)
    shotm_f = scr_pool.tile([NMAP, WAY], F32, name="shotm_f", tag="setup3")
    nc.gpsimd.memset(shotm_f[:], 1.0 / SHOT)
    nc.gpsimd.affine_select(
        out=shotm_f[:], in_=shotm_f[:], pattern=[[-SHOT, WAY]],
        compare_op=OP.is_ge, fill=0.0, base=0, channel_multiplier=1)
    nc.gpsimd.affine_select(
        out=shotm_f[:], in_=shotm_f[:], pattern=[[SHOT, WAY]],
        compare_op=OP.is_ge, fill=0.0, base=SHOT - 1, channel_multiplier=-1)
    shotm = const_pool.tile([NMAP, WAY], BF16, name="shotm", tag="shotm")
    nc.vector.tensor_copy(shotm[:], shotm_f[:])

    # identity for PE transposes (bf16)
    ident_f = scr_pool.tile([128, 128], F32, name="ident_f", tag="setup4")
    nc.gpsimd.memset(ident_f[:], 1.0)
    nc.gpsimd.affine_select(
        out=ident_f[:], in_=ident_f[:], pattern=[[-1, 128]],
        compare_op=OP.is_equal, fill=0.0, base=0, channel_multiplier=1)
    identb = const_pool.tile([128, 128], BF16, name="identb", tag="identb")
    nc.vector.tensor_copy(identb[:], ident_f[:])

    # rpn_w broadcast to all partitions via a k=1 matmul (bf16)
    ones1 = const_pool.tile([1, P], BF16, name="ones1", tag="ones1")
    nc.vector.memset(ones1[:], 1.0)
    wb = const_pool.tile([P, C], BF16, name="wb", tag="wb")
    for cc in range(5):
        sm_t = small_ps.tile([128, 1024], BF16, name=f"wbs{cc}", tag="smalls")
        wb_ps = sm_t[:, 512:768].bitcast(F32)
        nc.tensor.matmul(wb_ps[0:P, :], ones1[:], w_sb[:, 128 * cc:128 * (cc + 1)],
                         start=True, stop=True)
        nc.scalar.copy(wb[:, 128 * cc:128 * (cc + 1)], wb_ps[0:P, :])

    # final sim accumulator (all episodes)
    sim_all = const_pool.tile([QN, E * WAY], F32, name="sim_all", tag="sim_all")

    def rsqrt(out_ap, x_ap, n, tag):
        """out = 1/sqrt(x): bit-trick seed + 2 Newton iterations (DVE)."""
        y = stats.tile([P, n], F32, name=f"nw_y_{tag}", tag=f"nwy{n}")
        t = stats.tile([P, n], F32, name=f"nw_t_{tag}", tag=f"nwt{n}")
        nc.vector.tensor_scalar(y.bitcast(I32)[:], x_ap.bitcast(I32), 1, None,
                                op0=OP.arith_shift_right)
        nc.vector.tensor_scalar(y.bitcast(I32)[:], y.bitcast(I32)[:], -1,
                                0x5F3759DF, op0=OP.mult, op1=OP.add)
        for it in range(2):
            dst = out_ap if it == 1 else y[:]
            nc.vector.tensor_mul(t[:], y[:], y[:])
            nc.vector.tensor_mul(t[:], t[:], x_ap)
            nc.vector.tensor_scalar(t[:], t[:], -0.5, 1.5,
                                    op0=OP.mult, op1=OP.add)
            nc.vector.tensor_mul(dst, y[:], t[:])
        return out_ap

    def sq_pass(big, j, acc_col, method):
        src = big[:, C * j:C * (j + 1)]
        if method == "act":
            scr = scr_pool.tile([P, C], BF16, name="sq_a", tag="sq_a", bufs=3)
            nc.scalar.activation(scr[:], src, AF.Square, accum_out=acc_col)
        elif method == "tt":
            scr = scr_pool.tile([P, C], BF16, name="sq_t", tag="sq_t", bufs=3)
            nc.vector.tensor_tensor(out=scr[:], in0=src, in1=src, op=OP.mult)
            nc.vector.tensor_scalar(scr[:], scr[:], 1.0, 0.0, op0=OP.mult,
                                    op1=OP.add, accum_out=acc_col)
        else:
            scr = scr_pool.tile([P, C], BF16, name="sq_v", tag="sq_v", bufs=3)
            nc.vector.scalar_tensor_tensor(
                out=scr[:], in0=src, scalar=1.0, in1=src,
                op0=OP.mult, op1=OP.mult, accum_out=acc_col)

    # saved per-episode state between phases
    st_state = {}

    def phase_a(e):
        sbig, qbig = stiles[e], qtiles[e]
        # shared per-episode psum banks (see pool comment above)
        smalls = small_ps.tile([128, 1024], BF16, name=f"smalls_{e}",
                               tag="smalls")
        lo_bank = lo_ps.tile([128, NHI], F32, name=f"lo_{e}", tag="lo")

        # ---- support stats ----
        sn2 = stats.tile([P, ST], F32, name=f"sn2_{e}", tag="sn2")
        rr = stats.tile([P, ST], F32, name=f"rr_{e}", tag="rr")
        for j in range(ST):
            sq_pass(sbig, j, sn2[:, j:j + 1], SQ_METHOD[("s", j)])
            scr = scr_pool.tile([P, C], BF16, name="s_tt", tag="s_tt", bufs=3)
            nc.vector.tensor_tensor(out=scr[:], in0=sbig[:, C * j:C * (j + 1)],
                                    in1=wb[:], op=OP.mult)
            nc.vector.tensor_scalar(scr[:], scr[:], 1.0, 0.0, op0=OP.mult,
                                    op1=OP.add, accum_out=rr[:, j:j + 1])
        sinv = stats.tile([P, ST], F32, name=f"sinv_{e}", tag="sinv")
        rsqrt(sinv[:], sn2[:], ST, f"s{e % 2}")
        # softmax over hw within each map (logits tiny: no max-shift)
        lg = stats.tile([P, ST], F32, name=f"lg_{e}", tag="lg")
        nc.vector.tensor_mul(lg[:], rr[:], sinv[:])
        el = stats.tile([P, ST], BF16, name=f"el_{e}", tag="el")
        nc.scalar.activation(el[:], lg[:], AF.Exp)
        # per-map sums of exp via PE (lhsT = unweighted att mask blocks)
        sums = smalls[:, 384:386].bitcast(F32)[0:NMAP, :]
        for j in range(ST):
            nc.tensor.matmul(sums, stcv[:, j, 0, 0:NMAP],
                             el[:, j:j + 1], start=(j == 0), stop=(j == ST - 1))
        rec = stats.tile([NMAP, 1], F32, name=f"rec_{e}", tag="rec")
        nc.vector.reciprocal(rec[:], sums)
        uw = stats.tile([P, ST], F32, name=f"uw_{e}", tag="uw")
        nc.vector.tensor_mul(uw[:], el[:], sinv[:])
        # combined stationary: per-slot tensor_scalar (single-src 4x path;
        # broadcast APs would drop DVE to 1x)
        st_all = work.tile([P, ST, 2, NM2], BF16, name=f"st_{e}", tag="st_all")
        for j in range(ST):
            nc.vector.tensor_scalar_mul(st_all[:, j, 0], stcv[:, j, 0],
                                        uw[:, j:j + 1])
            nc.vector.tensor_scalar_mul(st_all[:, j, 1], stcv[:, j, 1],
                                        sinv[:, j:j + 1])
        # support reduce: one moving pass, both att-weighted and mean rows
        cgsm_hi = s_ps.tile([2 * NM2, NHI], F32, name=f"cgh_{e}", tag="cgh")
        cgsm_lo = lo_bank[0:2 * NM2, 128:256]
        for j in range(ST):
            lhs = st_all[:, j].rearrange("p t m -> p (t m)")
            nc.tensor.matmul(cgsm_hi[:], lhs, sbig[:, C * j:C * j + NHI],
                             start=(j == 0), stop=(j == ST - 1))
            nc.tensor.matmul(cgsm_lo[:], lhs, sbig[:, C * j + NHI:C * (j + 1)],
                             start=(j == 0), stop=(j == ST - 1))

        # ---- query stats ----
        qn2 = stats.tile([P, QT], F32, name=f"qn2_{e}", tag="qn2")
        for j in range(QT):
            sq_pass(qbig, j, qn2[:, j:j + 1], SQ_METHOD[("q", j)])
        qinv = stats.tile([P, QT], F32, name=f"qinv_{e}", tag="qinv")
        rsqrt(qinv[:], qn2[:], QT, f"q{e % 2}")
        sel_all = work.tile([P, QT, QNP], BF16, name=f"sel_{e}",
                            tag="sel_all")
        qmv = qmask[:].rearrange("p (j q) -> p j q", j=QT, q=QNP)
        for j in range(QT):
            nc.vector.tensor_scalar_mul(sel_all[:, j], qmv[:, j],
                                        qinv[:, j:j + 1])
        qm_hi = qm_ps.tile([QN, NHI], F32, name=f"qmh_{e}", tag="qmh")
        qm_lo = lo_bank[0:QN, 0:128]
        for j in range(QT):
            nc.tensor.matmul(qm_hi[:], sel_all[:, j, 0:QN],
                             qbig[:, C * j:C * j + NHI],
                             start=(j == 0), stop=(j == QT - 1))
            nc.tensor.matmul(qm_lo[:], sel_all[:, j, 0:QN],
                             qbig[:, C * j + NHI:C * (j + 1)],
                             start=(j == 0), stop=(j == QT - 1))
        st_state[e] = (cgsm_hi, cgsm_lo, qm_hi, qm_lo, rec, smalls)

    def phase_b(e):
        cgsm_hi, cgsm_lo, qm_hi, qm_lo, rec, smalls = st_state.pop(e)
        # gamma folded into the softmax normalizer (leaky commutes with
        # positive scaling)
        recg = stats.tile([NMAP, 1], F32, name=f"recg_{e}", tag="recg")
        nc.vector.tensor_scalar_mul(recg[:], rec[:], GAMMA)
        cg_sb = work.tile([NMAP, C], BF16, name=f"cg_{e}", tag="cg_sb")
        nc.scalar.activation(cg_sb[:, 0:NHI], cgsm_hi[0:NMAP, :], AF.Copy,
                             scale=recg[:, 0:1])
        nc.scalar.activation(cg_sb[:, NHI:C], cgsm_lo[0:NMAP, :], AF.Copy,
                             scale=recg[:, 0:1])
        sm_sb = work.tile([NMAP, C], BF16, name=f"sm_{e}", tag="sm_sb")
        nc.scalar.copy(sm_sb[:, 0:NHI], cgsm_hi[32:32 + NMAP, :])
        nc.scalar.copy(sm_sb[:, NHI:C], cgsm_lo[32:32 + NMAP, :])
        qm_sb = work.tile([QN, C], BF16, name=f"qm_{e}", tag="qm_sb")
        nc.scalar.copy(qm_sb[:, 0:NHI], qm_hi[:])
        nc.scalar.copy(qm_sb[:, NHI:C], qm_lo[:])

        # fp = max(slope*cg, cg) + sm   (cg already has gamma folded in)
        fpw = work.tile([NMAP, C], BF16, name=f"fpw_{e}", tag="fpw")
        nc.vector.scalar_tensor_tensor(
            out=fpw[:], in0=cg_sb[:], scalar=SLOPE, in1=cg_sb[:],
            op0=OP.mult, op1=OP.max)
        fp = work.tile([NMAP, C], BF16, name=f"fp_{e}", tag="fp")
        nc.vector.tensor_tensor(out=fp[:], in0=fpw[:], in1=sm_sb[:], op=OP.add)

        # featT [c-chunk, way] directly: lhsT = fp chunk, rhs = shotm
        featT = smalls[:, 388:438].bitcast(F32)
        for cc in range(5):
            nc.tensor.matmul(featT[:, WAY * cc:WAY * (cc + 1)],
                             fp[:, 128 * cc:128 * (cc + 1)], shotm[:],
                             start=True, stop=True)
        featT_sb = work.tile([128, WAY * 5], BF16, name=f"ft_{e}", tag="ftsb")
        nc.vector.tensor_copy(featT_sb[:], featT[:])

        # qmT via PE transposes into the same shared bank (bf16 region;
        # column stride 76 keeps each region 4-byte aligned in psum)
        qmT = smalls[:, 0:5 * 76]
        for cc in range(5):
            nc.tensor.transpose(qmT[:, 76 * cc:76 * cc + QN],
                                qm_sb[:, 128 * cc:128 * (cc + 1)],
                                identb[0:QN, 0:QN])
        qmT_sb = work.tile([128, 5 * 76], BF16, name=f"qmTs_{e}", tag="qmTs")
        nc.vector.tensor_copy(qmT_sb[:], qmT[:])

        sim_ps = smalls[:, 448:458].bitcast(F32)[0:QN, :]
        for cc in range(5):
            nc.tensor.matmul(sim_ps, qmT_sb[:, 76 * cc:76 * cc + QN],
                             featT_sb[:, WAY * cc:WAY * (cc + 1)],
                             start=(cc == 0), stop=(cc == 4))
        nc.vector.tensor_copy(sim_all[:, WAY * e:WAY * (e + 1)], sim_ps)

    # software pipeline: A0 A1 B0 A2 B1 A3 B2 B3
    phase_a(0)
    phase_a(1)
    phase_b(0)
    phase_a(2)
    phase_b(1)
    phase_a(3)
    phase_b(2)
    phase_b(3)

    nc.sync.dma_start(out, sim_all[:])


def build_program():
    nc = bacc.Bacc("TRN2", target_bir_lowering=False, debug=False,
                   num_devices=NCORES)
    inp1 = nc.dram_tensor("input1", [E, P, QT * C], BF16, kind="ExternalInput")
    inp2 = nc.dram_tensor("input2", [E, P, ST * C], BF16, kind="ExternalInput")
    rpnw = nc.dram_tensor("rpn_w", [1, C], BF16, kind="ExternalInput")
    out = nc.dram_tensor("sim", [QN, E * WAY], F32, kind="ExternalOutput")
    with tile.TileContext(nc) as tc, ExitStack() as ctx:
        _build_body(ctx, tc, inp1.ap(), inp2.ap(), rpnw.ap(), out.ap())
    nc.compile()
    return nc


_NC = None


def _get_nc():
    global _NC
    if _NC is None:
        _NC = build_program()
    return _NC


def shard_inputs(input1, input2, rpn_w, rpn_b=None):
    """Shard over episodes; relayout [E, 1875, 640] -> [E, 125, 15*640] is a
    pure reshape (descriptor d = 15p + j, slots consecutive in DRAM)."""
    bf = ml_dtypes.bfloat16
    i1 = np.asarray(input1, dtype=np.float32).reshape(B, P, QT * C).astype(bf)
    i2 = np.asarray(input2, dtype=np.float32).reshape(B, P, ST * C).astype(bf)
    w = np.asarray(rpn_w, dtype=np.float32).reshape(1, C).astype(bf)
    in_maps = []
    for i in range(NCORES):
        in_maps.append({
            "input1": np.ascontiguousarray(i1[E * i:E * (i + 1)]),
            "input2": np.ascontiguousarray(i2[E * i:E * (i + 1)]),
            "rpn_w": w,
        })
    return in_maps


def _ensure_ntff_hook():
    """Install the NTFF profile hook (the image's antenv lacks axon_hooks)."""
    import types
    import antenv

    if "antenv.axon_hooks" not in sys.modules:
        mod = types.ModuleType("antenv.axon_hooks")
        mod._hook = None
        mod.set_axon_ntff_profile_hook = lambda h: setattr(mod, "_hook", h)
        mod.get_axon_ntff_profile_hook = lambda: mod._hook
        sys.modules["antenv.axon_hooks"] = mod
        antenv.axon_hooks = mod
    mod = sys.modules["antenv.axon_hooks"]
    if mod.get_axon_ntff_profile_hook() is None:
        from trn_agent_boot.trn_boot import _ntff_profile_via_ctypes
        hook = _ntff_profile_via_ctypes("/opt/axon/libaxon_pjrt.so")
        if hook is not None:
            mod.set_axon_ntff_profile_hook(hook)


def kernel(input1, input2, rpn_w, rpn_b=None, **run_kwargs):
    if run_kwargs.get("trace"):
        _ensure_ntff_hook()
    nc = _get_nc()
    in_maps = shard_inputs(input1, input2, rpn_w)
    res = run_bass_kernel_spmd(nc, in_maps, list(range(NCORES)), **run_kwargs)
    out = np.concatenate(
        [r["sim"].reshape(QN, E, WAY).transpose(1, 0, 2) for r in res.results],
        axis=0)
    if run_kwargs:
        kernel.last_results = res
    return out.astype(np.float32)


# revision 27
# speedup vs baseline: 1.0128x; 1.0128x over previous
"""MetaBaseline (retrieval_knn) Trainium2 kernel — bf16 pipeline.

Computation (per episode b):
  q  = l2norm(input1[b])            # [75, 25, 640] over channel
  s  = l2norm(input2[b])            # [5, 5, 25, 640]
  att = softmax_hw(s @ rpn_w)       # rpn_b is softmax-invariant
  cg  = leaky(sum_hw(att * s))
  feat = mean_shot(mean_hw(s) + 5 * cg)
  sim[b] = mean_hw(q) @ feat.T      # [75, 5]

Sharding: data-parallel over episodes, 4 per core on 8 cores.

Layout: spatial descriptors on SBUF partitions (125/tile, descriptor
d = 15p + j for query, 5p + j for support), channels on the free axis.
All inputs are pre-converted to bf16 on the host (halves DMA bytes and
engages the DVE 16-bit perf modes; PE streams bf16 at 1 col/cycle).
Every group-reduction over descriptors (hw-mean, softmax sums, weighted
channel attention) is a PE matmul against a small stationary mask with
per-descriptor weights folded in. The final sim GEMM contracts over
channels via PE transposes of qm and a direct featT product.
All episode DMAs are issued up-front (whole per-core input fits SBUF).
"""

import sys
from contextlib import ExitStack

sys.path.insert(0, "/opt/trn_rl_repo")

import numpy as np
import ml_dtypes

import concourse.bass as bass
import concourse.tile as tile
from concourse import bacc, mybir
from concourse.bass_utils import run_bass_kernel_spmd

F32 = mybir.dt.float32
BF16 = mybir.dt.bfloat16
I32 = mybir.dt.int32
OP = mybir.AluOpType
AF = mybir.ActivationFunctionType

# Problem constants (fixed by the problem statement).
B, QN, WAY, SHOT, HH, WW, C = 32, 75, 5, 5, 5, 5, 640
NCORES = 8
E = B // NCORES        # 4 episodes per core
HW = HH * WW           # 25 spatial positions
P = 125                # descriptors per tile
QT = 15                # query slots per partition (1875 = 125*15)
ST = 5                 # support slots per partition (625 = 125*5)
NMAP = WAY * SHOT      # 25 support maps / episode
GAMMA = 5.0
SLOPE = 0.01
NHI = 512              # psum-bank-sized column split
NLO = C - NHI          # 128

# Square-pass method per tile: "pow" = DVE tensor_scalar x^2 + accum (4x
# candidate), "tt" = DVE tensor_tensor square into scratch + tensor_scalar
# reduce, "stt" = DVE scalar_tensor_tensor (1x), "act" = ACT Square,
# "gp" = gpsimd scalar_tensor_tensor.
# "stt" = DVE fused square+reduce (1x), "act" = ACT Square with
# accumulator, "acs" = ACT Square into a grouped scratch (reduced later by
# one batched DVE tensor_reduce at 2x), "gp" = gpsimd square into the same
# scratch.
SQ_S = ["stt", "act", "stt", "act", "stt"]
SQ_Q = ["stt", "stt", "act", "acs", "acs", "act", "gp", "gp", "gp",
        "act", "acs", "acs", "act", "stt", "stt"]


def _build_body(ctx: ExitStack, tc: "tile.TileContext", i1, i2, rpnw, out):
    nc = tc.nc

    const_pool = ctx.enter_context(tc.tile_pool(name="const", bufs=1))
    data_pool = ctx.enter_context(tc.tile_pool(name="data", bufs=1))
    scr_pool = ctx.enter_context(tc.tile_pool(name="scratch", bufs=1))
    stats = ctx.enter_context(tc.tile_pool(name="stats", bufs=2))
    work = ctx.enter_context(tc.tile_pool(name="work", bufs=2))

    # PSUM budget is 8 banks of [128, 512] f32. Per episode (double
    # buffered): qm_hi bank, cgsm_hi bank, a shared "lo" bank holding both
    # 128-col tails, and a shared "smalls" bank (bf16 tile; f32 regions are
    # bitcast views) holding softmax sums, featT, qmT and sim.
    qm_ps = ctx.enter_context(tc.tile_pool(name="qmps", bufs=2, space="PSUM"))
    s_ps = ctx.enter_context(tc.tile_pool(name="sps", bufs=2, space="PSUM"))
    lo_ps = ctx.enter_context(tc.tile_pool(name="lops", bufs=2, space="PSUM"))
    small_ps = ctx.enter_context(tc.tile_pool(name="smallps", bufs=2, space="PSUM"))

    # rpn_w first, on the scalar HWDGE ring (ahead of any bulk)
    w_sb = const_pool.tile([1, C], BF16, name="w_sb", tag="w_sb")
    nc.scalar.dma_start(w_sb[:], rpnw)

    # ================= all bulk DMAs up-front =================
    qtiles, stiles = [], []
    for e in range(E):
        sb = data_pool.tile([P, ST * C], BF16, name=f"s_{e}", tag=f"s_{e}")
        qb = data_pool.tile([P, QT * C], BF16, name=f"q_{e}", tag=f"q_{e}")
        stiles.append(sb)
        qtiles.append(qb)
    # Ring plan: SWDGE (gpsimd, all 16 SDMA engines) carries ~7.2MB as
    # 7.2KB descriptors; the two HWDGE rings (sync=SP, scalar=ACT; ~5
    # engines each) carry ~2.8MB each. rpn_w goes FIRST on the scalar ring
    # so the softmax chain never queues behind bulk (the v1 mistake).
    hw_rings = [nc.sync, nc.scalar]
    for e in range(E):
        hw_rings[e % 2].dma_start(stiles[e][:], i2[e])
        for cch in range(3):
            nc.gpsimd.dma_start(qtiles[e][:, 3200 * cch:3200 * (cch + 1)],
                                i1[e, :, 3200 * cch:3200 * (cch + 1)])

    # ================= one-time constants =================
    QNP = 76  # query mask column stride (pad 75 -> 76: keeps per-slot
    #           slices 4-byte aligned so DVE picks the 4x perf mode)
    setup_f32 = scr_pool.tile([P, QT * QNP], F32, name="setup_f32", tag="setup")

    # query mask, all 15 slots: [125, 15, 76], value 1/25 where
    # 0 <= 15p + j - 25q <= 24
    def stair(dst_f32, ncols, slots, j, value):
        nc.gpsimd.memset(dst_f32, value)
        nc.gpsimd.affine_select(
            out=dst_f32, in_=dst_f32, pattern=[[-HW, ncols]],
            compare_op=OP.is_ge, fill=0.0, base=j, channel_multiplier=slots)
        nc.gpsimd.affine_select(
            out=dst_f32, in_=dst_f32, pattern=[[HW, ncols]],
            compare_op=OP.is_ge, fill=0.0, base=HW - 1 - j,
            channel_multiplier=-slots)

    nc.gpsimd.memset(setup_f32[:], 0.0)
    for j in range(QT):
        stair(setup_f32[:, j * QNP:j * QNP + QN], QN, QT, j, 1.0 / HW)
    qmask = const_pool.tile([P, QT * QNP], BF16, name="qmask", tag="qmask")
    nc.vector.tensor_copy(qmask[:], setup_f32[:])

    # support combined mask [125, 5 slots, 2, 32]: att part (1.0) on
    # map-cols 0-24, hw-mean part (1/25) on cols 32-56 (pad to 32 so the
    # mean rows land on psum partition 32 -- PSUM reads must be 32-aligned).
    NM2 = 32
    setup2 = scr_pool.tile([P, ST * 2 * NM2], F32, name="setup2", tag="setup2")
    nc.gpsimd.memset(setup2[:], 0.0)
    s2v = setup2[:].rearrange("p (j t m) -> p j t m", j=ST, t=2, m=NM2)
    for j in range(ST):
        stair(s2v[:, j, 0, 0:NMAP], NMAP, ST, j, 1.0)
        stair(s2v[:, j, 1, 0:NMAP], NMAP, ST, j, 1.0 / HW)
    stc = const_pool.tile([P, ST * 2 * NM2], BF16, name="stc", tag="stc")
    nc.vector.tensor_copy(stc[:], setup2[:])
    stcv = stc[:].rearrange("p (j t m) -> p j t m", j=ST, t=2, m=NM2)

    # shot-mean matrix [25 maps, 5 ways] (block diagonal, 1/SHOT)
    shotm_f = scr_pool.tile([NMAP, WAY], F32, name="shotm_f", tag="setup3")
    nc.gpsimd.memset(shotm_f[:], 1.0 / SHOT)
    nc.gpsimd.affine_select(
        out=shotm_f[:], in_=shotm_f[:], pattern=[[-SHOT, WAY]],
        compare_op=OP.is_ge, fill=0.0, base=0, channel_multiplier=1)
    nc.gpsimd.affine_select(
        out=shotm_f[:], in_=shotm_f[:], pattern=[[SHOT, WAY]],
        compare_op=OP.is_ge, fill=0.0, base=SHOT - 1, channel_multiplier=-1)
    shotm = const_pool.tile([NMAP, WAY], BF16, name="shotm", tag="shotm")
    nc.vector.tensor_copy(shotm[:], shotm_f[:])

    # identity for PE transposes (bf16)
    ident_f = scr_pool.tile([128, 128], F32, name="ident_f", tag="setup4")
    nc.gpsimd.memset(ident_f[:], 1.0)
    nc.gpsimd.affine_select(
        out=ident_f[:], in_=ident_f[:], pattern=[[-1, 128]],
        compare_op=OP.is_equal, fill=0.0, base=0, channel_multiplier=1)
    identb = const_pool.tile([128, 128], BF16, name="identb", tag="identb")
    nc.vector.tensor_copy(identb[:], ident_f[:])

    # rpn_w broadcast to all partitions via a k=1 matmul (bf16)
    ones1 = const_pool.tile([1, P], BF16, name="ones1", tag="ones1")
    nc.vector.memset(ones1[:], 1.0)
    wb = const_pool.tile([P, C], BF16, name="wb", tag="wb")
    for cc in range(5):
        sm_t = small_ps.tile([128, 1024], BF16, name=f"wbs{cc}", tag="smalls")
        wb_ps = sm_t[:, 512:768].bitcast(F32)
        nc.tensor.matmul(wb_ps[0:P, :], ones1[:], w_sb[:, 128 * cc:128 * (cc + 1)],
                         start=True, stop=True)
        nc.scalar.copy(wb[:, 128 * cc:128 * (cc + 1)], wb_ps[0:P, :])

    # final sim accumulator (all episodes)
    sim_all = const_pool.tile([QN, E * WAY], F32, name="sim_all", tag="sim_all")

    def rsqrt(out_ap, x_ap, n, tag):
        """out = 1/sqrt(x): bit-trick seed + 1 Newton iteration (DVE).
        Seed error ~3.4%, after one iteration ~0.17% -- the error is random
        per descriptor and averages out over the 25-descriptor means, far
        inside the 2e-2 gate."""
        y = stats.tile([P, n], F32, name=f"nw_y_{tag}", tag=f"nwy{n}")
        t = stats.tile([P, n], F32, name=f"nw_t_{tag}", tag=f"nwt{n}")
        nc.vector.tensor_scalar(y.bitcast(I32)[:], x_ap.bitcast(I32), 1, None,
                                op0=OP.arith_shift_right)
        nc.vector.tensor_scalar(y.bitcast(I32)[:], y.bitcast(I32)[:], -1,
                                0x5F3759DF, op0=OP.mult, op1=OP.add)
        nc.vector.tensor_mul(t[:], y[:], y[:])
        nc.vector.tensor_mul(t[:], t[:], x_ap)
        nc.vector.tensor_scalar(t[:], t[:], -0.5, 1.5,
                                op0=OP.mult, op1=OP.add)
        nc.vector.tensor_mul(out_ap, y[:], t[:])
        return out_ap

    def sq_pass(big, j, acc_col, method, xx_slice):
        src = big[:, C * j:C * (j + 1)]
        if method == "act":
            scr = scr_pool.tile([P, C], BF16, name="sq_a", tag="sq_a", bufs=3)
            nc.scalar.activation(scr[:], src, AF.Square, accum_out=acc_col)
        elif method == "acs":
            nc.scalar.activation(xx_slice, src, AF.Square)
        elif method == "gp":
            nc.gpsimd.tensor_tensor(out=xx_slice, in0=src, in1=src,
                                    op=OP.mult)
        else:
            scr = scr_pool.tile([P, C], BF16, name="sq_v", tag="sq_v", bufs=3)
            nc.vector.scalar_tensor_tensor(
                out=scr[:], in0=src, scalar=1.0, in1=src,
                op0=OP.mult, op1=OP.mult, accum_out=acc_col)

    # saved per-episode state between phases
    st_state = {}

    def phase_a(e):
        sbig, qbig = stiles[e], qtiles[e]
        # shared per-episode psum banks (see pool comment above)
        smalls = small_ps.tile([128, 1024], BF16, name=f"smalls_{e}",
                               tag="smalls")
        lo_bank = lo_ps.tile([128, NHI], F32, name=f"lo_{e}", tag="lo")

        # ---- support stats ----
        sn2 = stats.tile([P, ST], F32, name=f"sn2_{e}", tag="sn2")
        for j in range(ST):
            sq_pass(sbig, j, sn2[:, j:j + 1], SQ_S[j], None)
        # logits products on gpsimd; one batched 2x tensor_reduce on DVE
        xw = scr_pool.tile([P, ST, C], BF16, name=f"xw_{e}", tag="xw", bufs=2)
        for j in range(ST):
            nc.gpsimd.tensor_tensor(out=xw[:, j], in0=sbig[:, C * j:C * (j + 1)],
                                    in1=wb[:], op=OP.mult)
        rr = stats.tile([P, ST], BF16, name=f"rr_{e}", tag="rr")
        with nc.allow_low_precision(reason="bf16 reduce out; fp32 internal"):
            nc.vector.tensor_reduce(rr[:], xw[:], axis=mybir.AxisListType.X,
                                    op=OP.add)
        sinv = stats.tile([P, ST], F32, name=f"sinv_{e}", tag="sinv")
        rsqrt(sinv[:], sn2[:], ST, f"s{e % 2}")
        # softmax over hw within each map (logits tiny: no max-shift)
        lg = stats.tile([P, ST], F32, name=f"lg_{e}", tag="lg")
        nc.vector.tensor_mul(lg[:], rr[:], sinv[:])
        el = stats.tile([P, ST], BF16, name=f"el_{e}", tag="el")
        nc.scalar.activation(el[:], lg[:], AF.Exp)
        # per-map sums of exp via PE (lhsT = unweighted att mask blocks)
        sums = smalls[:, 384:386].bitcast(F32)[0:NMAP, :]
        for j in range(ST):
            nc.tensor.matmul(sums, stcv[:, j, 0, 0:NMAP],
                             el[:, j:j + 1], start=(j == 0), stop=(j == ST - 1))
        rec = stats.tile([NMAP, 1], F32, name=f"rec_{e}", tag="rec")
        nc.vector.reciprocal(rec[:], sums)
        uw = stats.tile([P, ST], F32, name=f"uw_{e}", tag="uw")
        nc.vector.tensor_mul(uw[:], el[:], sinv[:])
        # weights [125, 5, 2] interleaved (uw_j, sinv_j); one broadcast
        # multiply builds the whole combined stationary
        w2 = stats.tile([P, ST, 2], F32, name=f"w2_{e}", tag="w2")
        nc.vector.tensor_copy(w2[:, :, 0], uw[:])
        nc.vector.tensor_copy(w2[:, :, 1], sinv[:])
        st_all = work.tile([P, ST, 2, NM2], BF16, name=f"st_{e}", tag="st_all")
        nc.vector.tensor_tensor(
            out=st_all[:], in0=stcv,
            in1=w2[:].unsqueeze(3).broadcast_to([P, ST, 2, NM2]),
            op=OP.mult)
        # support reduce: one moving pass, both att-weighted and mean rows
        cgsm_hi = s_ps.tile([2 * NM2, NHI], F32, name=f"cgh_{e}", tag="cgh")
        cgsm_lo = lo_bank[0:2 * NM2, 128:256]
        for j in range(ST):
            lhs = st_all[:, j].rearrange("p t m -> p (t m)")
            nc.tensor.matmul(cgsm_hi[:], lhs, sbig[:, C * j:C * j + NHI],
                             start=(j == 0), stop=(j == ST - 1))
            nc.tensor.matmul(cgsm_lo[:], lhs, sbig[:, C * j + NHI:C * (j + 1)],
                             start=(j == 0), stop=(j == ST - 1))

        # ---- query stats ----
        scr_slots = [j for j in range(QT) if SQ_Q[j] in ("acs", "gp")]
        NS = len(scr_slots)
        qn2 = stats.tile([P, QT], F32, name=f"qn2_{e}", tag="qn2")
        xq = scr_pool.tile([P, NS, C], BF16, name=f"xq_{e}", tag="xq", bufs=2)
        for j in range(QT):
            xsl = xq[:, scr_slots.index(j)] if j in scr_slots else None
            sq_pass(qbig, j, qn2[:, j:j + 1], SQ_Q[j], xsl)
        # one batched 2x reduce for all scratch slots, then cast into qn2
        qnb = stats.tile([P, NS], BF16, name=f"qnb_{e}", tag="qnb")
        with nc.allow_low_precision(reason="bf16 reduce out; fp32 internal"):
            nc.vector.tensor_reduce(qnb[:], xq[:], axis=mybir.AxisListType.X,
                                    op=OP.add)
        for j in scr_slots:
            nc.vector.tensor_copy(qn2[:, j:j + 1],
                                  qnb[:, scr_slots.index(j):
                                      scr_slots.index(j) + 1])
        qinv = stats.tile([P, QT], F32, name=f"qinv_{e}", tag="qinv")
        rsqrt(qinv[:], qn2[:], QT, f"q{e % 2}")
        qinv_bf = stats.tile([P, QT], BF16, name=f"qinvb_{e}", tag="qinvb")
        nc.vector.tensor_copy(qinv_bf[:], qinv[:])
        sel_all = work.tile([P, QT, QNP], BF16, name=f"sel_{e}",
                            tag="sel_all")
        qmv = qmask[:].rearrange("p (j q) -> p j q", j=QT, q=QNP)
        nc.gpsimd.tensor_tensor(
            out=sel_all[:], in0=qmv,
            in1=qinv_bf[:].unsqueeze(2).broadcast_to([P, QT, QNP]),
            op=OP.mult)
        qm_hi = qm_ps.tile([QN, NHI], F32, name=f"qmh_{e}", tag="qmh")
        qm_lo = lo_bank[0:QN, 0:128]
        for j in range(QT):
            nc.tensor.matmul(qm_hi[:], sel_all[:, j, 0:QN],
                             qbig[:, C * j:C * j + NHI],
                             start=(j == 0), stop=(j == QT - 1))
            nc.tensor.matmul(qm_lo[:], sel_all[:, j, 0:QN],
                             qbig[:, C * j + NHI:C * (j + 1)],
                             start=(j == 0), stop=(j == QT - 1))
        st_state[e] = (cgsm_hi, cgsm_lo, qm_hi, qm_lo, rec, smalls)

    def phase_b(e):
        cgsm_hi, cgsm_lo, qm_hi, qm_lo, rec, smalls = st_state.pop(e)
        # gamma folded into the softmax normalizer (leaky commutes with
        # positive scaling)
        recg = stats.tile([NMAP, 1], F32, name=f"recg_{e}", tag="recg")
        nc.vector.tensor_scalar_mul(recg[:], rec[:], GAMMA)
        # leaky folded into the evacuation: prelu(recg * cg, slope)
        cg_sb = work.tile([NMAP, C], BF16, name=f"cg_{e}", tag="cg_sb")
        nc.scalar.activation(cg_sb[:, 0:NHI], cgsm_hi[0:NMAP, :], AF.Prelu,
                             scale=recg[:, 0:1], alpha=SLOPE)
        nc.scalar.activation(cg_sb[:, NHI:C], cgsm_lo[0:NMAP, :], AF.Prelu,
                             scale=recg[:, 0:1], alpha=SLOPE)
        sm_sb = work.tile([NMAP, C], BF16, name=f"sm_{e}", tag="sm_sb")
        nc.scalar.copy(sm_sb[:, 0:NHI], cgsm_hi[32:32 + NMAP, :])
        nc.scalar.copy(sm_sb[:, NHI:C], cgsm_lo[32:32 + NMAP, :])
        qm_sb = work.tile([QN, C], BF16, name=f"qm_{e}", tag="qm_sb")
        nc.scalar.copy(qm_sb[:, 0:NHI], qm_hi[:])
        nc.scalar.copy(qm_sb[:, NHI:C], qm_lo[:])

        # fp = leaky(gamma*cg) + sm
        fp = work.tile([NMAP, C], BF16, name=f"fp_{e}", tag="fp")
        nc.vector.tensor_tensor(out=fp[:], in0=cg_sb[:], in1=sm_sb[:],
                                op=OP.add)

        # featT [c-chunk, way] directly: lhsT = fp chunk, rhs = shotm
        featT = smalls[:, 388:438].bitcast(F32)
        for cc in range(5):
            nc.tensor.matmul(featT[:, WAY * cc:WAY * (cc + 1)],
                             fp[:, 128 * cc:128 * (cc + 1)], shotm[:],
                             start=True, stop=True)
        featT_sb = work.tile([128, WAY * 5], BF16, name=f"ft_{e}", tag="ftsb")
        nc.vector.tensor_copy(featT_sb[:], featT[:])

        # qmT via PE transposes into the same shared bank (bf16 region;
        # column stride 76 keeps each region 4-byte aligned in psum)
        qmT = smalls[:, 0:5 * 76]
        for cc in range(5):
            nc.tensor.transpose(qmT[:, 76 * cc:76 * cc + QN],
                                qm_sb[:, 128 * cc:128 * (cc + 1)],
                                identb[0:QN, 0:QN])
        qmT_sb = work.tile([128, 5 * 76], BF16, name=f"qmTs_{e}", tag="qmTs")
        nc.vector.tensor_copy(qmT_sb[:], qmT[:])

        sim_ps = smalls[:, 448:458].bitcast(F32)[0:QN, :]
        for cc in range(5):
            nc.tensor.matmul(sim_ps, qmT_sb[:, 76 * cc:76 * cc + QN],
                             featT_sb[:, WAY * cc:WAY * (cc + 1)],
                             start=(cc == 0), stop=(cc == 4))
        nc.vector.tensor_copy(sim_all[:, WAY * e:WAY * (e + 1)], sim_ps)

    # software pipeline: A0 A1 B0 A2 B1 A3 B2 B3
    phase_a(0)
    phase_a(1)
    phase_b(0)
    phase_a(2)
    phase_b(1)
    phase_a(3)
    phase_b(2)
    phase_b(3)

    nc.sync.dma_start(out, sim_all[:])


def build_program():
    nc = bacc.Bacc("TRN2", target_bir_lowering=False, debug=False,
                   num_devices=NCORES)
    inp1 = nc.dram_tensor("input1", [E, P, QT * C], BF16, kind="ExternalInput")
    inp2 = nc.dram_tensor("input2", [E, P, ST * C], BF16, kind="ExternalInput")
    rpnw = nc.dram_tensor("rpn_w", [1, C], BF16, kind="ExternalInput")
    out = nc.dram_tensor("sim", [QN, E * WAY], F32, kind="ExternalOutput")
    with tile.TileContext(nc) as tc, ExitStack() as ctx:
        _build_body(ctx, tc, inp1.ap(), inp2.ap(), rpnw.ap(), out.ap())
    nc.compile()
    return nc


_NC = None


def _get_nc():
    global _NC
    if _NC is None:
        _NC = build_program()
    return _NC


def shard_inputs(input1, input2, rpn_w, rpn_b=None):
    """Shard over episodes; relayout [E, 1875, 640] -> [E, 125, 15*640] is a
    pure reshape (descriptor d = 15p + j, slots consecutive in DRAM)."""
    bf = ml_dtypes.bfloat16
    i1 = np.asarray(input1, dtype=np.float32).reshape(B, P, QT * C).astype(bf)
    i2 = np.asarray(input2, dtype=np.float32).reshape(B, P, ST * C).astype(bf)
    w = np.asarray(rpn_w, dtype=np.float32).reshape(1, C).astype(bf)
    in_maps = []
    for i in range(NCORES):
        in_maps.append({
            "input1": np.ascontiguousarray(i1[E * i:E * (i + 1)]),
            "input2": np.ascontiguousarray(i2[E * i:E * (i + 1)]),
            "rpn_w": w,
        })
    return in_maps


def _ensure_ntff_hook():
    """Install the NTFF profile hook (the image's antenv lacks axon_hooks)."""
    import types
    import antenv

    if "antenv.axon_hooks" not in sys.modules:
        mod = types.ModuleType("antenv.axon_hooks")
        mod._hook = None
        mod.set_axon_ntff_profile_hook = lambda h: setattr(mod, "_hook", h)
        mod.get_axon_ntff_profile_hook = lambda: mod._hook
        sys.modules["antenv.axon_hooks"] = mod
        antenv.axon_hooks = mod
    mod = sys.modules["antenv.axon_hooks"]
    if mod.get_axon_ntff_profile_hook() is None:
        from trn_agent_boot.trn_boot import _ntff_profile_via_ctypes
        hook = _ntff_profile_via_ctypes("/opt/axon/libaxon_pjrt.so")
        if hook is not None:
            mod.set_axon_ntff_profile_hook(hook)


def kernel(input1, input2, rpn_w, rpn_b=None, **run_kwargs):
    if run_kwargs.get("trace"):
        _ensure_ntff_hook()
    nc = _get_nc()
    in_maps = shard_inputs(input1, input2, rpn_w)
    res = run_bass_kernel_spmd(nc, in_maps, list(range(NCORES)), **run_kwargs)
    out = np.concatenate(
        [r["sim"].reshape(QN, E, WAY).transpose(1, 0, 2) for r in res.results],
        axis=0)
    if run_kwargs:
        kernel.last_results = res
    return out.astype(np.float32)


# revision 28
# speedup vs baseline: 1.1477x; 1.1332x over previous
"""MetaBaseline (retrieval_knn) Trainium2 kernel — bf16 pipeline.

Computation (per episode b):
  q  = l2norm(input1[b])            # [75, 25, 640] over channel
  s  = l2norm(input2[b])            # [5, 5, 25, 640]
  att = softmax_hw(s @ rpn_w)       # rpn_b is softmax-invariant
  cg  = leaky(sum_hw(att * s))
  feat = mean_shot(mean_hw(s) + 5 * cg)
  sim[b] = mean_hw(q) @ feat.T      # [75, 5]

Sharding: data-parallel over episodes, 4 per core on 8 cores.

Layout: spatial descriptors on SBUF partitions (125/tile, descriptor
d = 15p + j for query, 5p + j for support), channels on the free axis.
All inputs are pre-converted to bf16 on the host (halves DMA bytes and
engages the DVE 16-bit perf modes; PE streams bf16 at 1 col/cycle).
Every group-reduction over descriptors (hw-mean, softmax sums, weighted
channel attention) is a PE matmul against a small stationary mask with
per-descriptor weights folded in. The final sim GEMM contracts over
channels via PE transposes of qm and a direct featT product.
All episode DMAs are issued up-front (whole per-core input fits SBUF).
"""

import sys
from contextlib import ExitStack

sys.path.insert(0, "/opt/trn_rl_repo")

import numpy as np
import ml_dtypes

import concourse.bass as bass
import concourse.tile as tile
from concourse import bacc, mybir
from concourse.bass_utils import run_bass_kernel_spmd

F32 = mybir.dt.float32
BF16 = mybir.dt.bfloat16
I32 = mybir.dt.int32
OP = mybir.AluOpType
AF = mybir.ActivationFunctionType

# Problem constants (fixed by the problem statement).
B, QN, WAY, SHOT, HH, WW, C = 32, 75, 5, 5, 5, 5, 640
NCORES = 8
E = B // NCORES        # 4 episodes per core
HW = HH * WW           # 25 spatial positions
P = 125                # descriptors per tile
QT = 15                # query slots per partition (1875 = 125*15)
ST = 5                 # support slots per partition (625 = 125*5)
NMAP = WAY * SHOT      # 25 support maps / episode
GAMMA = 5.0
SLOPE = 0.01
NHI = 512              # psum-bank-sized column split
NLO = C - NHI          # 128

# Square-pass method per tile: "pow" = DVE tensor_scalar x^2 + accum (4x
# candidate), "tt" = DVE tensor_tensor square into scratch + tensor_scalar
# reduce, "stt" = DVE scalar_tensor_tensor (1x), "act" = ACT Square,
# "gp" = gpsimd scalar_tensor_tensor.
# "stt" = DVE fused square+reduce (1x), "act" = ACT Square with
# accumulator, "acs" = ACT Square into a grouped scratch (reduced later by
# one batched DVE tensor_reduce at 2x), "gp" = gpsimd square into the same
# scratch.
SQ_S = ["stt", "act", "stt", "act", "stt"]
SQ_Q = ["stt", "act", "stt", "act", "stt", "act", "stt", "act", "stt",
        "act", "stt", "act", "stt", "act", "stt"]


def _build_body(ctx: ExitStack, tc: "tile.TileContext", i1, i2, rpnw, out):
    nc = tc.nc

    const_pool = ctx.enter_context(tc.tile_pool(name="const", bufs=1))
    data_pool = ctx.enter_context(tc.tile_pool(name="data", bufs=1))
    scr_pool = ctx.enter_context(tc.tile_pool(name="scratch", bufs=1))
    stats = ctx.enter_context(tc.tile_pool(name="stats", bufs=2))
    work = ctx.enter_context(tc.tile_pool(name="work", bufs=2))

    # PSUM budget is 8 banks of [128, 512] f32. Per episode (double
    # buffered): qm_hi bank, cgsm_hi bank, a shared "lo" bank holding both
    # 128-col tails, and a shared "smalls" bank (bf16 tile; f32 regions are
    # bitcast views) holding softmax sums, featT, qmT and sim.
    qm_ps = ctx.enter_context(tc.tile_pool(name="qmps", bufs=2, space="PSUM"))
    s_ps = ctx.enter_context(tc.tile_pool(name="sps", bufs=2, space="PSUM"))
    lo_ps = ctx.enter_context(tc.tile_pool(name="lops", bufs=2, space="PSUM"))
    small_ps = ctx.enter_context(tc.tile_pool(name="smallps", bufs=2, space="PSUM"))

    # rpn_w first, on the scalar HWDGE ring (ahead of any bulk)
    w_sb = const_pool.tile([1, C], BF16, name="w_sb", tag="w_sb")
    nc.scalar.dma_start(w_sb[:], rpnw)

    # ================= all bulk DMAs up-front =================
    qtiles, stiles = [], []
    for e in range(E):
        sb = data_pool.tile([P, ST * C], BF16, name=f"s_{e}", tag=f"s_{e}")
        qb = data_pool.tile([P, QT * C], BF16, name=f"q_{e}", tag=f"q_{e}")
        stiles.append(sb)
        qtiles.append(qb)
    # Ring plan: SWDGE (gpsimd, all 16 SDMA engines) carries ~7.2MB as
    # 7.2KB descriptors; the two HWDGE rings (sync=SP, scalar=ACT; ~5
    # engines each) carry ~2.8MB each. rpn_w goes FIRST on the scalar ring
    # so the softmax chain never queues behind bulk (the v1 mistake).
    hw_rings = [nc.sync, nc.scalar]
    for e in range(E):
        hw_rings[e % 2].dma_start(stiles[e][:], i2[e])
        for cch in range(3):
            nc.gpsimd.dma_start(qtiles[e][:, 3200 * cch:3200 * (cch + 1)],
                                i1[e, :, 3200 * cch:3200 * (cch + 1)])

    # ================= one-time constants =================
    QNP = 76  # query mask column stride (pad 75 -> 76: keeps per-slot
    #           slices 4-byte aligned so DVE picks the 4x perf mode)
    setup_f32 = scr_pool.tile([P, QT * QNP], F32, name="setup_f32", tag="setup")

    # query mask, all 15 slots: [125, 15, 76], value 1/25 where
    # 0 <= 15p + j - 25q <= 24
    def stair(dst_f32, ncols, slots, j, value):
        nc.gpsimd.memset(dst_f32, value)
        nc.gpsimd.affine_select(
            out=dst_f32, in_=dst_f32, pattern=[[-HW, ncols]],
            compare_op=OP.is_ge, fill=0.0, base=j, channel_multiplier=slots)
        nc.gpsimd.affine_select(
            out=dst_f32, in_=dst_f32, pattern=[[HW, ncols]],
            compare_op=OP.is_ge, fill=0.0, base=HW - 1 - j,
            channel_multiplier=-slots)

    nc.gpsimd.memset(setup_f32[:], 0.0)
    for j in range(QT):
        stair(setup_f32[:, j * QNP:j * QNP + QN], QN, QT, j, 1.0 / HW)
    qmask = const_pool.tile([P, QT * QNP], BF16, name="qmask", tag="qmask")
    nc.vector.tensor_copy(qmask[:], setup_f32[:])

    # support combined mask [125, 5 slots, 2, 32]: att part (1.0) on
    # map-cols 0-24, hw-mean part (1/25) on cols 32-56 (pad to 32 so the
    # mean rows land on psum partition 32 -- PSUM reads must be 32-aligned).
    NM2 = 32
    setup2 = scr_pool.tile([P, ST * 2 * NM2], F32, name="setup2", tag="setup2")
    nc.gpsimd.memset(setup2[:], 0.0)
    s2v = setup2[:].rearrange("p (j t m) -> p j t m", j=ST, t=2, m=NM2)
    for j in range(ST):
        stair(s2v[:, j, 0, 0:NMAP], NMAP, ST, j, 1.0)
        stair(s2v[:, j, 1, 0:NMAP], NMAP, ST, j, 1.0 / HW)
    stc = const_pool.tile([P, ST * 2 * NM2], BF16, name="stc", tag="stc")
    nc.vector.tensor_copy(stc[:], setup2[:])
    stcv = stc[:].rearrange("p (j t m) -> p j t m", j=ST, t=2, m=NM2)

    # shot-mean matrix [25 maps, 5 ways] (block diagonal, 1/SHOT)
    shotm_f = scr_pool.tile([NMAP, WAY], F32, name="shotm_f", tag="setup3")
    nc.gpsimd.memset(shotm_f[:], 1.0 / SHOT)
    nc.gpsimd.affine_select(
        out=shotm_f[:], in_=shotm_f[:], pattern=[[-SHOT, WAY]],
        compare_op=OP.is_ge, fill=0.0, base=0, channel_multiplier=1)
    nc.gpsimd.affine_select(
        out=shotm_f[:], in_=shotm_f[:], pattern=[[SHOT, WAY]],
        compare_op=OP.is_ge, fill=0.0, base=SHOT - 1, channel_multiplier=-1)
    shotm = const_pool.tile([NMAP, WAY], BF16, name="shotm", tag="shotm")
    nc.vector.tensor_copy(shotm[:], shotm_f[:])

    # identity for PE transposes (bf16)
    ident_f = scr_pool.tile([128, 128], F32, name="ident_f", tag="setup4")
    nc.gpsimd.memset(ident_f[:], 1.0)
    nc.gpsimd.affine_select(
        out=ident_f[:], in_=ident_f[:], pattern=[[-1, 128]],
        compare_op=OP.is_equal, fill=0.0, base=0, channel_multiplier=1)
    identb = const_pool.tile([128, 128], BF16, name="identb", tag="identb")
    nc.vector.tensor_copy(identb[:], ident_f[:])

    # rpn_w broadcast to all partitions via a k=1 matmul (bf16)
    ones1 = const_pool.tile([1, P], BF16, name="ones1", tag="ones1")
    nc.vector.memset(ones1[:], 1.0)
    wb = const_pool.tile([P, C], BF16, name="wb", tag="wb")
    for cc in range(5):
        sm_t = small_ps.tile([128, 1024], BF16, name=f"wbs{cc}", tag="smalls")
        wb_ps = sm_t[:, 512:768].bitcast(F32)
        nc.tensor.matmul(wb_ps[0:P, :], ones1[:], w_sb[:, 128 * cc:128 * (cc + 1)],
                         start=True, stop=True)
        nc.scalar.copy(wb[:, 128 * cc:128 * (cc + 1)], wb_ps[0:P, :])

    # final sim accumulator (all episodes)
    sim_all = const_pool.tile([QN, E * WAY], F32, name="sim_all", tag="sim_all")

    def rsqrt(out_ap, x_ap, n, tag):
        """out = 1/sqrt(x): bit-trick seed + 1 Newton iteration (DVE).
        Seed error ~3.4%, after one iteration ~0.17% -- the error is random
        per descriptor and averages out over the 25-descriptor means, far
        inside the 2e-2 gate."""
        y = stats.tile([P, n], F32, name=f"nw_y_{tag}", tag=f"nwy{n}")
        t = stats.tile([P, n], F32, name=f"nw_t_{tag}", tag=f"nwt{n}")
        nc.vector.tensor_scalar(y.bitcast(I32)[:], x_ap.bitcast(I32), 1, None,
                                op0=OP.arith_shift_right)
        nc.vector.tensor_scalar(y.bitcast(I32)[:], y.bitcast(I32)[:], -1,
                                0x5F3759DF, op0=OP.mult, op1=OP.add)
        nc.vector.tensor_mul(t[:], y[:], y[:])
        nc.vector.tensor_mul(t[:], t[:], x_ap)
        nc.vector.tensor_scalar(t[:], t[:], -0.5, 1.5,
                                op0=OP.mult, op1=OP.add)
        nc.vector.tensor_mul(out_ap, y[:], t[:])
        return out_ap

    def sq_pass(big, j, acc_col, method, xx_slice):
        src = big[:, C * j:C * (j + 1)]
        if method == "act":
            scr = scr_pool.tile([P, C], BF16, name="sq_a", tag="sq_a", bufs=3)
            nc.scalar.activation(scr[:], src, AF.Square, accum_out=acc_col)
        elif method == "acs":
            nc.scalar.activation(xx_slice, src, AF.Square)
        elif method == "gp":
            nc.gpsimd.tensor_tensor(out=xx_slice, in0=src, in1=src,
                                    op=OP.mult)
        else:
            scr = scr_pool.tile([P, C], BF16, name="sq_v", tag="sq_v", bufs=3)
            nc.vector.scalar_tensor_tensor(
                out=scr[:], in0=src, scalar=1.0, in1=src,
                op0=OP.mult, op1=OP.mult, accum_out=acc_col)

    # saved per-episode state between phases
    st_state = {}

    def phase_a(e):
        sbig, qbig = stiles[e], qtiles[e]
        # shared per-episode psum banks (see pool comment above)
        smalls = small_ps.tile([128, 1024], BF16, name=f"smalls_{e}",
                               tag="smalls")
        lo_bank = lo_ps.tile([128, NHI], F32, name=f"lo_{e}", tag="lo")

        # ---- support stats ----
        sn2 = stats.tile([P, ST], F32, name=f"sn2_{e}", tag="sn2")
        for j in range(ST):
            sq_pass(sbig, j, sn2[:, j:j + 1], SQ_S[j], None)
        # logits products on gpsimd; one batched 2x tensor_reduce on DVE
        xw = scr_pool.tile([P, ST, C], BF16, name=f"xw_{e}", tag="xw", bufs=2)
        for j in range(ST):
            nc.gpsimd.tensor_tensor(out=xw[:, j], in0=sbig[:, C * j:C * (j + 1)],
                                    in1=wb[:], op=OP.mult)
        rr = stats.tile([P, ST], BF16, name=f"rr_{e}", tag="rr")
        with nc.allow_low_precision(reason="bf16 reduce out; fp32 internal"):
            nc.vector.tensor_reduce(rr[:], xw[:], axis=mybir.AxisListType.X,
                                    op=OP.add)
        sinv = stats.tile([P, ST], F32, name=f"sinv_{e}", tag="sinv")
        rsqrt(sinv[:], sn2[:], ST, f"s{e % 2}")
        # softmax over hw within each map (logits tiny: no max-shift)
        lg = stats.tile([P, ST], F32, name=f"lg_{e}", tag="lg")
        nc.vector.tensor_mul(lg[:], rr[:], sinv[:])
        el = stats.tile([P, ST], BF16, name=f"el_{e}", tag="el")
        nc.scalar.activation(el[:], lg[:], AF.Exp)
        # per-map sums of exp via PE (lhsT = unweighted att mask blocks)
        sums = smalls[:, 384:386].bitcast(F32)[0:NMAP, :]
        for j in range(ST):
            nc.tensor.matmul(sums, stcv[:, j, 0, 0:NMAP],
                             el[:, j:j + 1], start=(j == 0), stop=(j == ST - 1))
        rec = stats.tile([NMAP, 1], F32, name=f"rec_{e}", tag="rec")
        nc.vector.reciprocal(rec[:], sums)
        uw = stats.tile([P, ST], F32, name=f"uw_{e}", tag="uw")
        nc.vector.tensor_mul(uw[:], el[:], sinv[:])
        # weights [125, 5, 2] interleaved (uw_j, sinv_j); one broadcast
        # multiply builds the whole combined stationary
        w2 = stats.tile([P, ST, 2], F32, name=f"w2_{e}", tag="w2")
        nc.vector.tensor_copy(w2[:, :, 0], uw[:])
        nc.vector.tensor_copy(w2[:, :, 1], sinv[:])
        st_all = work.tile([P, ST, 2, NM2], BF16, name=f"st_{e}", tag="st_all")
        nc.gpsimd.tensor_tensor(
            out=st_all[:], in0=stcv,
            in1=w2[:].unsqueeze(3).broadcast_to([P, ST, 2, NM2]),
            op=OP.mult)
        # support reduce: one moving pass, both att-weighted and mean rows
        cgsm_hi = s_ps.tile([2 * NM2, NHI], F32, name=f"cgh_{e}", tag="cgh")
        cgsm_lo = lo_bank[0:2 * NM2, 128:256]
        for j in range(ST):
            lhs = st_all[:, j].rearrange("p t m -> p (t m)")
            nc.tensor.matmul(cgsm_hi[:], lhs, sbig[:, C * j:C * j + NHI],
                             start=(j == 0), stop=(j == ST - 1))
            nc.tensor.matmul(cgsm_lo[:], lhs, sbig[:, C * j + NHI:C * (j + 1)],
                             start=(j == 0), stop=(j == ST - 1))

        # ---- query stats ----
        qn2 = stats.tile([P, QT], F32, name=f"qn2_{e}", tag="qn2")
        for j in range(QT):
            sq_pass(qbig, j, qn2[:, j:j + 1], SQ_Q[j], None)
        qinv = stats.tile([P, QT], F32, name=f"qinv_{e}", tag="qinv")
        rsqrt(qinv[:], qn2[:], QT, f"q{e % 2}")
        qinv_bf = stats.tile([P, QT], BF16, name=f"qinvb_{e}", tag="qinvb")
        nc.vector.tensor_copy(qinv_bf[:], qinv[:])
        sel_all = work.tile([P, QT, QNP], BF16, name=f"sel_{e}",
                            tag="sel_all")
        qmv = qmask[:].rearrange("p (j q) -> p j q", j=QT, q=QNP)
        nc.gpsimd.tensor_tensor(
            out=sel_all[:], in0=qmv,
            in1=qinv_bf[:].unsqueeze(2).broadcast_to([P, QT, QNP]),
            op=OP.mult)
        qm_hi = qm_ps.tile([QN, NHI], F32, name=f"qmh_{e}", tag="qmh")
        qm_lo = lo_bank[0:QN, 0:128]
        for j in range(QT):
            nc.tensor.matmul(qm_hi[:], sel_all[:, j, 0:QN],
                             qbig[:, C * j:C * j + NHI],
                             start=(j == 0), stop=(j == QT - 1))
            nc.tensor.matmul(qm_lo[:], sel_all[:, j, 0:QN],
                             qbig[:, C * j + NHI:C * (j + 1)],
                             start=(j == 0), stop=(j == QT - 1))
        st_state[e] = (cgsm_hi, cgsm_lo, qm_hi, qm_lo, rec, smalls)

    def phase_b(e):
        cgsm_hi, cgsm_lo, qm_hi, qm_lo, rec, smalls = st_state.pop(e)
        # gamma folded into the softmax normalizer (leaky commutes with
        # positive scaling)
        recg = stats.tile([NMAP, 1], F32, name=f"recg_{e}", tag="recg")
        nc.vector.tensor_scalar_mul(recg[:], rec[:], GAMMA)
        # leaky folded into the evacuation: prelu(recg * cg, slope)
        cg_sb = work.tile([NMAP, C], BF16, name=f"cg_{e}", tag="cg_sb")
        nc.scalar.activation(cg_sb[:, 0:NHI], cgsm_hi[0:NMAP, :], AF.Prelu,
                             scale=recg[:, 0:1], alpha=SLOPE)
        nc.scalar.activation(cg_sb[:, NHI:C], cgsm_lo[0:NMAP, :], AF.Prelu,
                             scale=recg[:, 0:1], alpha=SLOPE)
        sm_sb = work.tile([NMAP, C], BF16, name=f"sm_{e}", tag="sm_sb")
        nc.scalar.copy(sm_sb[:, 0:NHI], cgsm_hi[32:32 + NMAP, :])
        nc.scalar.copy(sm_sb[:, NHI:C], cgsm_lo[32:32 + NMAP, :])
        qm_sb = work.tile([QN, C], BF16, name=f"qm_{e}", tag="qm_sb")
        nc.vector.tensor_copy(qm_sb[:, 0:NHI], qm_hi[:])
        nc.vector.tensor_copy(qm_sb[:, NHI:C], qm_lo[:])

        # fp = leaky(gamma*cg) + sm
        fp = work.tile([NMAP, C], BF16, name=f"fp_{e}", tag="fp")
        nc.gpsimd.tensor_tensor(out=fp[:], in0=cg_sb[:], in1=sm_sb[:],
                                op=OP.add)

        # featT [c-chunk, way] directly: lhsT = fp chunk, rhs = shotm
        featT = smalls[:, 388:438].bitcast(F32)
        for cc in range(5):
            nc.tensor.matmul(featT[:, WAY * cc:WAY * (cc + 1)],
                             fp[:, 128 * cc:128 * (cc + 1)], shotm[:],
                             start=True, stop=True)
        featT_sb = work.tile([128, WAY * 5], BF16, name=f"ft_{e}", tag="ftsb")
        nc.vector.tensor_copy(featT_sb[:], featT[:])

        # qmT via PE transposes into the same shared bank (bf16 region;
        # column stride 76 keeps each region 4-byte aligned in psum)
        qmT = smalls[:, 0:5 * 76]
        for cc in range(5):
            nc.tensor.transpose(qmT[:, 76 * cc:76 * cc + QN],
                                qm_sb[:, 128 * cc:128 * (cc + 1)],
                                identb[0:QN, 0:QN])
        qmT_sb = work.tile([128, 5 * 76], BF16, name=f"qmTs_{e}", tag="qmTs")
        nc.vector.tensor_copy(qmT_sb[:], qmT[:])

        sim_ps = smalls[:, 448:458].bitcast(F32)[0:QN, :]
        for cc in range(5):
            nc.tensor.matmul(sim_ps, qmT_sb[:, 76 * cc:76 * cc + QN],
                             featT_sb[:, WAY * cc:WAY * (cc + 1)],
                             start=(cc == 0), stop=(cc == 4))
        nc.vector.tensor_copy(sim_all[:, WAY * e:WAY * (e + 1)], sim_ps)

    # software pipeline: A0 A1 B0 A2 B1 A3 B2 B3
    phase_a(0)
    phase_a(1)
    phase_b(0)
    phase_a(2)
    phase_b(1)
    phase_a(3)
    phase_b(2)
    phase_b(3)

    nc.sync.dma_start(out, sim_all[:])


def build_program():
    nc = bacc.Bacc("TRN2", target_bir_lowering=False, debug=False,
                   num_devices=NCORES)
    inp1 = nc.dram_tensor("input1", [E, P, QT * C], BF16, kind="ExternalInput")
    inp2 = nc.dram_tensor("input2", [E, P, ST * C], BF16, kind="ExternalInput")
    rpnw = nc.dram_tensor("rpn_w", [1, C], BF16, kind="ExternalInput")
    out = nc.dram_tensor("sim", [QN, E * WAY], F32, kind="ExternalOutput")
    with tile.TileContext(nc) as tc, ExitStack() as ctx:
        _build_body(ctx, tc, inp1.ap(), inp2.ap(), rpnw.ap(), out.ap())
    nc.compile()
    return nc


_NC = None


def _get_nc():
    global _NC
    if _NC is None:
        _NC = build_program()
    return _NC


def shard_inputs(input1, input2, rpn_w, rpn_b=None):
    """Shard over episodes; relayout [E, 1875, 640] -> [E, 125, 15*640] is a
    pure reshape (descriptor d = 15p + j, slots consecutive in DRAM)."""
    bf = ml_dtypes.bfloat16
    i1 = np.asarray(input1, dtype=np.float32).reshape(B, P, QT * C).astype(bf)
    i2 = np.asarray(input2, dtype=np.float32).reshape(B, P, ST * C).astype(bf)
    w = np.asarray(rpn_w, dtype=np.float32).reshape(1, C).astype(bf)
    in_maps = []
    for i in range(NCORES):
        in_maps.append({
            "input1": np.ascontiguousarray(i1[E * i:E * (i + 1)]),
            "input2": np.ascontiguousarray(i2[E * i:E * (i + 1)]),
            "rpn_w": w,
        })
    return in_maps


def _ensure_ntff_hook():
    """Install the NTFF profile hook (the image's antenv lacks axon_hooks)."""
    import types
    import antenv

    if "antenv.axon_hooks" not in sys.modules:
        mod = types.ModuleType("antenv.axon_hooks")
        mod._hook = None
        mod.set_axon_ntff_profile_hook = lambda h: setattr(mod, "_hook", h)
        mod.get_axon_ntff_profile_hook = lambda: mod._hook
        sys.modules["antenv.axon_hooks"] = mod
        antenv.axon_hooks = mod
    mod = sys.modules["antenv.axon_hooks"]
    if mod.get_axon_ntff_profile_hook() is None:
        from trn_agent_boot.trn_boot import _ntff_profile_via_ctypes
        hook = _ntff_profile_via_ctypes("/opt/axon/libaxon_pjrt.so")
        if hook is not None:
            mod.set_axon_ntff_profile_hook(hook)


def kernel(input1, input2, rpn_w, rpn_b=None, **run_kwargs):
    if run_kwargs.get("trace"):
        _ensure_ntff_hook()
    nc = _get_nc()
    in_maps = shard_inputs(input1, input2, rpn_w)
    res = run_bass_kernel_spmd(nc, in_maps, list(range(NCORES)), **run_kwargs)
    out = np.concatenate(
        [r["sim"].reshape(QN, E, WAY).transpose(1, 0, 2) for r in res.results],
        axis=0)
    if run_kwargs:
        kernel.last_results = res
    return out.astype(np.float32)
